# revision 6
# baseline (speedup 1.0000x reference)
"""Grouped per-task GEMM (multi-head routing) on 8 Trainium2 cores.

pred[i] = W[t[i]] @ x[i] + b[t[i]],  x:[B,D] f32, t:[B] int, W:[T,C,D], b:[T,C]
B=16384, D=1024, T=20, C=100.

Strategy (data-parallel, replicated weights, host-side routing):
  * Host: stable-sort samples by task id -> contiguous per-task runs of
    128-sample tiles. Runs are cut into groups of npg=4 tiles (last group of
    a run zero-padded), groups dealt round-robin across the 8 cores. Every
    tile in a group shares one task, so the device loads that task's weights
    ONCE per group, and a group's outputs fill exactly one PSUM bank.
  * Per group the host gathers the task's weights into a PE-ready transposed
    layout [128, 8*100] (partition = d-within-chunk, free = (k-chunk, class));
    x rows go to [128, 8*128] per tile. Everything DMAs as single fully
    contiguous transfers, cast to bf16 (fp32 accumulate in PSUM; ~3e-3
    scale-relative error, far inside the fp32-envelope gate).
  * Device (single static SPMD program), per group: weight DMA [128,800];
    4x { x DMA [128,1024]; 8 accumulating matmuls -> PSUM [100, bi*128:...] };
    one DVE bias-add [100,512] PSUM->SBUF; one contiguous [100,512] store.
  * Host: scatter tile columns back to pred via the sort permutation.
"""

import os
import numpy as np

B, D, T, C = 16384, 1024, 20, 100
NCORES = 8
P = 128          # samples per tile / rows per d-chunk
KC = D // P      # 8 contraction chunks
NPG = 4          # tiles per group == matmul outputs per PSUM bank

DTYPE = os.environ.get("KERNEL_DTYPE", "bf16")   # "bf16" | "f32"

_PROGRAM_CACHE = {}
LAST_RESULTS = None  # BassKernelResults of the most recent run (for profiling)


def _np_dtype():
    if DTYPE == "bf16":
        import ml_dtypes

        return np.dtype(ml_dtypes.bfloat16)
    return np.dtype(np.float32)


def build_program(g_max, npg=NPG, repeat=1):
    """One SPMD Tile program: g_max weight groups x npg tiles per core.
    `repeat` re-emits the body (benchmarking only)."""
    import concourse.bacc as bacc
    import concourse.mybir as mybir
    from concourse import tile

    f32 = mybir.dt.float32
    dt_in = mybir.dt.bfloat16 if DTYPE == "bf16" else f32
    npc = g_max * npg
    nc = bacc.Bacc(
        "TRN2", target_bir_lowering=False, debug=False, num_devices=NCORES
    )
    x_d = nc.dram_tensor("xh", [npc, P, D], dt_in, kind="ExternalInput").ap()
    w_d = nc.dram_tensor("wh", [g_max, P, KC * C], dt_in, kind="ExternalInput").ap()
    b_d = nc.dram_tensor("bh", [C, g_max], f32, kind="ExternalInput").ap()
    y_d = nc.dram_tensor("yh", [g_max, C, npg * P], f32, kind="ExternalOutput").ap()

    with tile.TileContext(nc) as tc:
        with (
            tc.tile_pool(name="xp", bufs=8) as xp,
            tc.tile_pool(name="wp", bufs=2) as wp,
            tc.tile_pool(name="cp", bufs=1) as cp,
            tc.tile_pool(name="op", bufs=3) as op,
            tc.tile_pool(name="pp", bufs=4, space="PSUM") as pp,
        ):
            b_sb = cp.tile([C, g_max], f32)
            nc.sync.dma_start(b_sb[:], b_d[:])
            for _ in range(repeat):
                for g in range(g_max):
                    wt = wp.tile([P, KC * C], dt_in)
                    nc.sync.dma_start(wt[:], w_d[g])
                    ps = pp.tile([C, npg * P], f32)
                    for bi in range(npg):
                        xt = xp.tile([P, D], dt_in)
                        nc.sync.dma_start(xt[:], x_d[g * npg + bi])
                        for k in range(KC):
                            nc.tensor.matmul(
                                ps[:, bi * P:(bi + 1) * P],
                                wt[:, k * C:(k + 1) * C],
                                xt[:, k * P:(k + 1) * P],
                                start=(k == 0),
                                stop=(k == KC - 1),
                            )
                    yo = op.tile([C, npg * P], f32)
                    nc.vector.tensor_scalar_add(yo[:], ps[:], b_sb[:, g:g + 1])
                    nc.sync.dma_start(y_d[g], yo[:])
    nc.compile()
    return nc


def _plan(t):
    """Sort+route plan. Returns (tasks [NCORES*g_max] core-major, src
    [NCORES*g_max, NPG*P] original-row indices (-1 = pad), g_max)."""
    counts = np.bincount(t, minlength=T)
    order = np.argsort(t, kind="stable")
    groups = np.split(order, np.cumsum(counts)[:-1])

    gtasks = []
    gsrc = []
    for tau in range(T):
        g = groups[tau]
        for s in range(0, len(g), NPG * P):
            chunk = g[s:s + NPG * P]
            rows = np.full(NPG * P, -1, dtype=np.int64)
            rows[: len(chunk)] = chunk
            gtasks.append(tau)
            gsrc.append(rows)
    n_groups = max(len(gtasks), 1)
    g_max = -(-n_groups // NCORES)
    g_total = g_max * NCORES
    tasks = np.full(g_total, -1, dtype=np.int64)
    src = np.full((g_total, NPG * P), -1, dtype=np.int64)
    if gtasks:
        tasks[: len(gtasks)] = gtasks
        src[: len(gtasks)] = np.stack(gsrc)
    # deal round-robin (group i -> core i%8), reorder core-major
    perm = np.arange(g_total).reshape(g_max, NCORES).T.reshape(-1)
    return tasks[perm], src[perm], g_max


def kernel(x, t, W, b):
    global LAST_RESULTS
    from concourse import bass_utils

    x = np.ascontiguousarray(np.asarray(x, dtype=np.float32))
    t = np.asarray(t).astype(np.int64, copy=False)
    W = np.ascontiguousarray(np.asarray(W, dtype=np.float32))
    b = np.ascontiguousarray(np.asarray(b, dtype=np.float32))
    dt_in = _np_dtype()

    tasks, src, g_max = _plan(t)     # core-major
    npc = g_max * NPG
    n_total = npc * NCORES

    # ---- gather x into [cores, npc, 128(p), 8(k)*128(s)] ----
    xg = np.zeros((n_total * P, D), dtype=np.float32)
    flat_src = src.reshape(-1)
    valid = flat_src >= 0
    xg[valid] = x[flat_src[valid]]
    xh = np.ascontiguousarray(
        xg.reshape(NCORES, npc, P, KC, P)   # (core, tile, s, k, p)
        .transpose(0, 1, 4, 3, 2)           # (core, tile, p, k, s)
        .reshape(NCORES, npc, P, D)
        .astype(dt_in)
    )

    # ---- per-task weights in [128(p), 8(k)*100(c)] layout, per group ----
    Wt = np.ascontiguousarray(
        W.transpose(0, 2, 1)                # [T, D, C]
        .reshape(T, KC, P, C)               # (task, k, p, c)
        .transpose(0, 2, 1, 3)              # (task, p, k, c)
        .reshape(T, P, KC * C)
        .astype(dt_in)
    )
    wh = np.zeros((NCORES * g_max, P, KC * C), dtype=dt_in)
    tvalid = tasks >= 0
    wh[tvalid] = Wt[tasks[tvalid]]
    wh = wh.reshape(NCORES, g_max, P, KC * C)

    bg = np.zeros((NCORES * g_max, C), dtype=np.float32)
    bg[tvalid] = b[tasks[tvalid]]
    bh = np.ascontiguousarray(
        bg.reshape(NCORES, g_max, C).transpose(0, 2, 1)
    )  # [core, C, g_max]

    # ---- compile (cached) + run ----
    key = (g_max, DTYPE)
    nc = _PROGRAM_CACHE.get(key)
    if nc is None:
        nc = build_program(g_max)
        _PROGRAM_CACHE[key] = nc

    in_maps = [
        {"xh": xh[m], "wh": wh[m], "bh": bh[m]} for m in range(NCORES)
    ]
    res = bass_utils.run_bass_kernel_spmd(
        nc, in_maps, core_ids=list(range(NCORES))
    )
    LAST_RESULTS = res

    # ---- unshard: scatter group columns back through the permutation ----
    pred = np.zeros((B, C), dtype=np.float32)
    for m in range(NCORES):
        y = np.asarray(res.results[m]["yh"])          # [g_max, C, NPG*P]
        rows = y.transpose(0, 2, 1).reshape(npc * P, C)
        s = src.reshape(NCORES, npc * P)[m]
        ok = s >= 0
        pred[s[ok]] = rows[ok]
    return pred
